# revision 31
# baseline (speedup 1.0000x reference)
"""GraphTransformer (2x PyG TransformerConv + out proj) on 8 trn2 NeuronCores.

v3 strategy (edge-parallel dst-ownership, DVE-lean edge math):
- Host: sort nodes by (degree, id); rank r -> core r%8, local r//8. Edges
  grouped by dst; per-tile max degree D_t is SPMD-uniform. Slot layout
  (group, core, local) makes the L2 AllGather chunkable into G groups.
- Layer 1 k|v table: each core builds kv only for the ~31.7k nodes its
  edges actually reference (host-compacted xTs input) -> smaller PE work
  and HBM traffic vs a full replicated table. Gather indices are
  subset-local.
- Layer 2: per-tile q,k,v,s projections inside the L1 edge loop; per-group
  bf16 AllGathers of the full k|v table overlap remaining L1 edge work.
- Edge phase per 128-dst tile: per-slot indirect gathers (bf16 kv row,
  2KB) round-robined over 2 SWDGE queues; logits via contiguous 2x-mode
  DVE ops (q broadcast in1, no materialized copy); additive -30000 mask
  bias pre-exp (robust to garbage in padded slots); exp weights expanded
  across C on the Act engine so the weighted-v product stays in DVE 2x
  mode with natural-layout writes; message accumulated per tile with one
  strided reduce (no transposed-write products, which measured 13-19us).
"""
import numpy as np

N, E, D, H, C, HC = 50000, 400000, 384, 4, 128, 512
NCORES, P = 8, 128
NLOC = N // NCORES
G = 7                     # collective groups (NTILES % G == 0)
NTILES = -(-((NLOC + P - 1) // P) // G) * G   # ceil to multiple of G
SHARD = NTILES * P
NPAD = SHARD * NCORES
SCHUNK = 12
NQ = 4                    # SWDGE queues for indirect gathers
TPG = NTILES // G         # tiles per group
RPG = TPG * P             # rows per group per core
INV_SQRT_C = 1.0 / np.sqrt(np.float32(C))
MASKNEG = -30000.0


# ---------------------------------------------------------------- host prep
def _prep(edge_index):
    src = np.asarray(edge_index[0], dtype=np.int64)
    dst = np.asarray(edge_index[1], dtype=np.int64)
    deg = np.bincount(dst, minlength=N)
    node_of_rank = np.lexsort((np.arange(N), deg))
    rank_of_node = np.empty(N, np.int64)
    rank_of_node[node_of_rank] = np.arange(N)
    # rank -> (core, local); local -> (group, j); table slot layout
    # s = (g*NCORES + c)*RPG + j  with g = local//RPG, j = local%RPG
    r = np.arange(N)
    core_r = r % NCORES
    local_r = r // NCORES
    slot_r = (local_r // RPG * NCORES + core_r) * RPG + local_r % RPG
    slot_of_node = np.empty(N, np.int64)
    slot_of_node[node_of_rank] = slot_r

    deg_sorted = deg[node_of_rank]
    Dts = []
    for t in range(NTILES):
        blk = deg_sorted[t * P * NCORES:(t + 1) * P * NCORES]
        Dts.append(max(int(blk.max()) if len(blk) else 0, 1))
    SUMD = sum(Dts)
    coloff = np.cumsum([0] + Dts)[:-1]

    er = rank_of_node[dst]
    order = np.argsort(er, kind="stable")
    er_s = er[order]
    gslot_s = slot_of_node[src[order]]
    starts = np.searchsorted(er_s, np.arange(N))
    slot = np.arange(E) - starts[er_s]

    core_e = er_s % NCORES
    local_e = er_s // NCORES
    col_e = coloff[local_e // P] + slot
    p_e = local_e % P

    srcidx1 = np.zeros((NCORES, P, SUMD), np.int32)   # subset-local (L1)
    srcidx2 = np.zeros((NCORES, P, SUMD), np.int32)   # global slot (L2)
    maskb = np.full((NCORES, P, SUMD), MASKNEG, np.float32)
    subsets = []
    for c in range(NCORES):
        m = core_e == c
        u = np.unique(gslot_s[m])
        sub_of = np.zeros(NPAD, np.int32)
        sub_of[u] = np.arange(len(u), dtype=np.int32)
        srcidx1[c, p_e[m], col_e[m]] = sub_of[gslot_s[m]]
        srcidx2[c, p_e[m], col_e[m]] = gslot_s[m]
        maskb[c, p_e[m], col_e[m]] = 0.0
        subsets.append(u)
    SUBPAD = -(-max(len(u) for u in subsets) // 1024) * 1024
    maskbH = np.repeat(maskb, H, axis=2)              # (s h) layout
    return (srcidx1, srcidx2, maskbH, Dts, SUMD, coloff,
            node_of_rank, slot_of_node, subsets, SUBPAD)


# ---------------------------------------------------------------- wait fix
def _split_waits(nc):
    """walrus here rejects >1 sem-wait per instruction; split extras onto
    InstNoOp carriers inserted just before, same engine."""
    import concourse.mybir as mybir
    for fn in nc.m.functions:
        for bb in fn.blocks:
            out = []
            changed = False
            for ins in bb.instructions:
                si = ins.sync_info
                waits = list(si.on_wait) if si and si.on_wait else []
                if len(waits) > 1:
                    changed = True
                    for j, w in enumerate(waits[:-1]):
                        out.append(mybir.InstNoOp(
                            name=f"{ins.name}-wf{j}", opcode="NoOp",
                            engine=ins.engine,
                            sync_info=mybir.SyncInfo(on_wait=[w], on_update=[]),
                            text_hint="waitfix"))
                    si.on_wait = waits[-1:]
                out.append(ins)
            if changed:
                bb.instructions = out


# ---------------------------------------------------------------- bass build
def _build_nc(Dts, SUMD, coloff, SUBPAD, bias_zero=False):
    import concourse.bass as bass
    import concourse.mybir as mybir
    import concourse.tile as tile
    from concourse.masks import make_identity
    f32 = mybir.dt.float32
    bf16 = mybir.dt.bfloat16

    nc = bass.Bass(num_devices=NCORES, num_swdge_queues=NQ)
    # L2 k|v table: raw Shared tensor so G chunked AllGathers can write
    # disjoint row ranges (pool tiles enforce a single writer). Ordering vs
    # the L2 gathers is by a manual semaphore incremented per AG.
    kvt1 = nc.dram_tensor("kvt1buf", [NPAD, 2 * HC], bf16,
                          kind="Internal", addr_space="Shared")
    agsem = nc.alloc_semaphore("agsem")
    xTs = nc.dram_tensor("xTs", [D, SUBPAD], bf16, kind="ExternalInput")
    xTo = nc.dram_tensor("xTo", [D, SHARD], bf16, kind="ExternalInput")
    srcidx1_d = nc.dram_tensor("srcidx1", [P, SUMD], mybir.dt.int32, kind="ExternalInput")
    srcidx2_d = nc.dram_tensor("srcidx2", [P, SUMD], mybir.dt.int32, kind="ExternalInput")
    maskb_d = nc.dram_tensor("maskb", [P, SUMD * H], bf16, kind="ExternalInput")
    wkv0 = nc.dram_tensor("wkv0", [D, 2 * HC], bf16, kind="ExternalInput")
    wqs0 = nc.dram_tensor("wqs0", [D, 2 * HC], bf16, kind="ExternalInput")
    wall1 = nc.dram_tensor("wall1", [HC, 4 * HC], bf16, kind="ExternalInput")
    wout = nc.dram_tensor("wout", [HC, D], bf16, kind="ExternalInput")
    bkv0 = nc.dram_tensor("bkv0", [1, 2 * HC], bf16, kind="ExternalInput")
    bqs0 = nc.dram_tensor("bqs0", [1, 2 * HC], bf16, kind="ExternalInput")
    ball1 = nc.dram_tensor("ball1", [1, 4 * HC], bf16, kind="ExternalInput")
    bout = nc.dram_tensor("bout", [1, D], bf16, kind="ExternalInput")
    out_d = nc.dram_tensor("out", [SHARD, D], f32, kind="ExternalOutput")

    chunks = []  # per tile: list of (coloff, S)
    for t in range(NTILES):
        cs, off = [], 0
        while off < Dts[t]:
            cs.append((int(coloff[t]) + off, min(SCHUNK, Dts[t] - off)))
            off += SCHUNK
        chunks.append(cs)
    ncmax = max(len(c) for c in chunks)

    KB0 = D // P   # 3
    KB1 = HC // P  # 4
    rg = [list(range(NCORES))]
    qrot = [0]

    def igather(out_ap, tab, off_ap):
        bi = nc.gpsimd.indirect_dma_start(
            out=out_ap, out_offset=None, in_=tab,
            in_offset=bass.IndirectOffsetOnAxis(ap=off_ap, axis=0))
        q = qrot[0] % NQ
        qrot[0] += 1
        if q:
            bi.ins.queue = f"qPoolDynamic{q}"
        return bi

    with tile.TileContext(nc) as tc:
        with (
            tc.tile_pool(name="dram", bufs=1, space="DRAM") as dram,
            tc.tile_pool(name="const", bufs=1) as const,
        ):
            kvt0 = dram.tile([SUBPAD, 2 * HC], bf16, name="kvt0")
            qsd = [dram.tile([SHARD, 2 * HC], bf16, name=f"qs{l}d")
                   for l in range(2)]
            kvin2 = dram.tile([SHARD, 2 * HC], bf16, name="kvin2")

            nc.gpsimd.sem_clear(range(agsem.num, agsem.num + 1))
            ident = const.tile([P, P], bf16)
            make_identity(nc, ident[:])
            ones = const.tile([1, P], bf16)
            nc.vector.memset(ones[:], 1.0)
            srcidx1_s = const.tile([P, SUMD], mybir.dt.int32)
            nc.sync.dma_start(srcidx1_s[:], srcidx1_d[:])
            srcidx2_s = const.tile([P, SUMD], mybir.dt.int32)
            nc.sync.dma_start(srcidx2_s[:], srcidx2_d[:])
            maskb_s = const.tile([P, SUMD * H], bf16)
            nc.sync.dma_start(maskb_s[:], maskb_d[:])

            w_kv0 = const.tile([P, KB0 * 2 * HC], bf16)
            nc.sync.dma_start(
                w_kv0[:].rearrange("p (kb n) -> p kb n", n=2 * HC),
                wkv0[:].rearrange("(kb p) n -> p kb n", p=P))
            w_qs0 = const.tile([P, KB0 * 2 * HC], bf16)
            nc.sync.dma_start(
                w_qs0[:].rearrange("p (kb n) -> p kb n", n=2 * HC),
                wqs0[:].rearrange("(kb p) n -> p kb n", p=P))
            w_all1 = const.tile([P, KB1 * 4 * HC], bf16)
            nc.sync.dma_start(
                w_all1[:].rearrange("p (kb n) -> p kb n", n=4 * HC),
                wall1[:].rearrange("(kb p) n -> p kb n", p=P))
            w_out = const.tile([P, KB1 * D], bf16)
            nc.sync.dma_start(
                w_out[:].rearrange("p (kb n) -> p kb n", n=D),
                wout[:].rearrange("(kb p) n -> p kb n", p=P))

            # biases, broadcast across partitions via ones-matmul (PE)
            bsrc = {}
            for nm, dt_, wid in (("bkv0", bkv0, 2 * HC), ("bqs0", bqs0, 2 * HC),
                                 ("ball1", ball1, 4 * HC), ("bout", bout, D)):
                t_ = const.tile([1, wid], bf16, name=f"{nm}row")
                nc.sync.dma_start(t_[:], dt_[:])
                bsrc[nm] = t_
            bb = {}
            with tc.tile_pool(name="bbp", bufs=1, space="PSUM") as bbp:
                psb = bbp.tile([P, 4 * HC], f32, tag="psb")
                for nm, wid in (("bkv0", 2 * HC), ("bqs0", 2 * HC),
                                ("ball1", 4 * HC), ("bout", D)):
                    for j0 in range(0, wid, HC):
                        j1 = min(j0 + HC, wid)
                        nc.tensor.matmul(psb[:, j0:j1], lhsT=ones[:1, :],
                                         rhs=bsrc[nm][:1, j0:j1],
                                         start=True, stop=True)
                    bb[nm] = const.tile([P, wid], bf16, name=f"bb{nm}")
                    nc.scalar.copy(bb[nm][:], psb[:, :wid])

            def evac(idx, o, ps, bname, scalar_only=False):
                """PSUM -> SBUF with bias add. gpsimd can't read PSUM on
                hw; alternate Act/DVE in PE-bound phases, Act-only during
                the edge phases (DVE is the bottleneck there)."""
                if bias_zero:
                    if scalar_only or idx % 2 == 0:
                        nc.scalar.copy(o, ps)
                    else:
                        nc.vector.tensor_copy(o, ps)
                else:
                    nc.vector.tensor_tensor(out=o, in0=ps, in1=bb[bname][:],
                                            op=mybir.AluOpType.add)

            # ---------------- layer-1 q,s projections (own shard) ----------
            def l1_qs():
                with (
                    tc.tile_pool(name="qsl", bufs=6) as lp,
                    tc.tile_pool(name="qso", bufs=2) as op,
                    tc.tile_pool(name="qsp", bufs=2, space="PSUM") as pp,
                ):
                    for g8 in range((NTILES + 7) // 8):
                        lhs = []
                        t0 = g8 * 8
                        nt = min(8, NTILES - t0)
                        for kb in range(KB0):
                            lt = lp.tile([P, 8 * P], bf16, tag="lhs")
                            nc.sync.dma_start(
                                lt[:, :nt * P],
                                xTo[kb * P:(kb + 1) * P, t0 * P:(t0 + nt) * P])
                            lhs.append(lt)
                        for ti in range(nt):
                            t = t0 + ti
                            ps = pp.tile([P, 2 * HC], f32, tag="ps")
                            for kb in range(KB0):
                                for j0 in range(0, 2 * HC, HC):
                                    nc.tensor.matmul(
                                        ps[:, j0:j0 + HC],
                                        lhsT=lhs[kb][:, ti * P:(ti + 1) * P],
                                        rhs=w_qs0[:].rearrange(
                                            "p (kb n) -> p kb n",
                                            n=2 * HC)[:, kb, j0:j0 + HC],
                                        start=(kb == 0), stop=(kb == KB0 - 1))
                            o = op.tile([P, 2 * HC], bf16, tag="o")
                            evac(t, o[:], ps[:], "bqs0")
                            rows = slice(t * P, (t + 1) * P)
                            nc.scalar.dma_start(qsd[0][rows, :], o[:])

            # ---------------- layer-1 subset k|v table ---------------------
            def l1_table():
                NT_F = SUBPAD // P
                with (
                    tc.tile_pool(name="tbl", bufs=6) as lp,
                    tc.tile_pool(name="tbo", bufs=3) as op,
                    tc.tile_pool(name="tbp", bufs=3, space="PSUM") as pp,
                ):
                    for g8 in range(NT_F // 8):
                        lhs = []
                        t0 = g8 * 8
                        for kb in range(KB0):
                            lt = lp.tile([P, 8 * P], bf16, tag="lhs")
                            nc.sync.dma_start(
                                lt[:], xTs[kb * P:(kb + 1) * P, t0 * P:(t0 + 8) * P])
                            lhs.append(lt)
                        for ti in range(8):
                            u = t0 + ti
                            ps = pp.tile([P, 2 * HC], f32, tag="ps")
                            for kb in range(KB0):
                                for j0 in range(0, 2 * HC, HC):
                                    nc.tensor.matmul(
                                        ps[:, j0:j0 + HC],
                                        lhsT=lhs[kb][:, ti * P:(ti + 1) * P],
                                        rhs=w_kv0[:].rearrange(
                                            "p (kb n) -> p kb n",
                                            n=2 * HC)[:, kb, j0:j0 + HC],
                                        start=(kb == 0), stop=(kb == KB0 - 1))
                            o = op.tile([P, 2 * HC], bf16, tag="o")
                            evac(u, o[:], ps[:], "bkv0")
                            qdma = nc.sync if u % 2 == 0 else nc.scalar
                            qdma.dma_start(kvt0[u * P:(u + 1) * P, :], o[:])

            # ---------------- edge phase (layer l), single pass ------------
            def edge_phase(l, hook):
                tab = kvt0 if l == 0 else kvt1
                sidx = srcidx1_s if l == 0 else srcidx2_s
                with (
                    tc.tile_pool(name=f"eg{l}", bufs=2) as eg,
                    tc.tile_pool(name=f"ew{l}", bufs=2) as ew,
                    tc.tile_pool(name=f"eh{l}", bufs=2) as eh,
                    tc.tile_pool(name=f"eT{l}", bufs=2) as eT,
                    tc.tile_pool(name=f"et{l}", bufs=2, space="PSUM") as et,
                ):
                    for t in range(NTILES):
                        rows = slice(t * P, (t + 1) * P)
                        Dt = Dts[t]
                        nch = len(chunks[t])
                        q_t = ew.tile([P, HC], bf16, tag="q")
                        nc.sync.dma_start(q_t[:], qsd[l][rows, 0:HC])
                        s_t = ew.tile([P, HC], bf16, tag="s")
                        nc.scalar.dma_start(s_t[:], qsd[l][rows, HC:2 * HC])
                        msgb = eh.tile([P, ncmax * SCHUNK * HC], bf16, tag="msgb")
                        e_all = eh.tile([P, ncmax * SCHUNK * H], bf16, tag="eall")
                        nc.vector.memset(e_all[:], 0.0)
                        for ci, (co, S) in enumerate(chunks[t]):
                            cb = ci * SCHUNK
                            kvg = eg.tile([P, SCHUNK * 2 * HC], bf16, tag="kvg")
                            if t < 2:
                                # first use of each ring buffer: stale SBUF
                                # bits could be NaN; NaN*0 != 0 in the padded
                                # ev slots below
                                nc.vector.memset(kvg[:], 0.0)
                            kv3 = kvg[:].rearrange("p (s kv) -> p s kv", kv=2 * HC)
                            for s in range(S):
                                igather(kv3[:, s, :], tab[:],
                                        sidx[:, co + s:co + s + 1])
                            # q*k product lands in msgb's chunk columns (later
                            # overwritten by the weighted-v product)
                            prod = msgb[:, cb * HC:(cb + SCHUNK) * HC]
                            nc.vector.tensor_tensor(
                                out=prod.rearrange(
                                    "p (s n) -> p s n", n=HC)[:, :S],
                                in0=kv3[:, :S, 0:HC],
                                in1=q_t[:, None, :].to_broadcast([P, S, HC]),
                                op=mybir.AluOpType.mult)
                            # logit reduction over C: two 2x contiguous folds
                            # (128->64->32) + one 1x reduce of 32. The fold
                            # scratch aliases the eexp tile (dead until the
                            # broadcast-copy below, which runs after the TR).
                            eexp = eT.tile([P, SCHUNK * HC], bf16, tag="eexp")
                            p3 = prod.rearrange("p (sh c) -> p sh c", c=C)
                            f3 = eexp[:, :SCHUNK * H * 64].rearrange(
                                "p (sh c) -> p sh c", c=64)
                            with nc.allow_low_precision(reason="bf16 edge math"):
                                nc.vector.tensor_tensor(
                                    out=f3[:, :S * H], in0=p3[:, :S * H, 0:64],
                                    in1=p3[:, :S * H, 64:128],
                                    op=mybir.AluOpType.add)
                                nc.vector.tensor_tensor(
                                    out=f3[:, :S * H, 0:32],
                                    in0=f3[:, :S * H, 0:32],
                                    in1=f3[:, :S * H, 32:64],
                                    op=mybir.AluOpType.add)
                                am = eT.tile([P, SCHUNK * H], bf16, tag="am")
                                nc.vector.tensor_reduce(
                                    am[:, :S * H],
                                    f3[:, :S * H, 0:32],
                                    axis=mybir.AxisListType.X,
                                    op=mybir.AluOpType.add)
                            nc.vector.tensor_tensor(
                                out=am[:, :S * H], in0=am[:, :S * H],
                                in1=maskb_s[:, (co) * H:(co + S) * H],
                                op=mybir.AluOpType.add)
                            nc.scalar.activation(
                                e_all[:, cb * H:cb * H + S * H],
                                am[:, :S * H],
                                mybir.ActivationFunctionType.Exp,
                                scale=float(INV_SQRT_C))
                            # e expanded across C on Act (has slack) so the
                            # weighted-v product stays a packed 2x DVE op
                            nc.scalar.copy(
                                eexp[:].rearrange(
                                    "p (s h c) -> p s h c", h=H, c=C),
                                e_all[:, cb * H:(cb + SCHUNK) * H]
                                .rearrange("p (s h) -> p s h", h=H)
                                [:, :, :, None].to_broadcast(
                                    [P, SCHUNK, H, C]))
                            # full SCHUNK width: padded slots have e==0 so the
                            # stale k|v garbage is zeroed, keeping the fold
                            # chain below valid
                            nc.vector.tensor_tensor(
                                out=msgb[:, cb * HC:(cb + SCHUNK) * HC],
                                in0=kv3[:, :, HC:2 * HC],
                                in1=eexp[:],
                                op=mybir.AluOpType.mult)
                            if ci > 0:
                                # fold this chunk's slots into block 0 now so
                                # the tile tail only folds one chunk's worth
                                with nc.allow_low_precision(
                                        reason="bf16 edge math"):
                                    nc.vector.tensor_tensor(
                                        out=msgb[:, :SCHUNK * HC],
                                        in0=msgb[:, :SCHUNK * HC],
                                        in1=msgb[:, cb * HC:
                                                 (cb + SCHUNK) * HC],
                                        op=mybir.AluOpType.add)
                        den = eh.tile([P, H], f32, tag="den")
                        nc.vector.tensor_reduce(
                            den[:],
                            e_all[:, :Dt * H].rearrange(
                                "p (s h) -> p h s", h=H),
                            axis=mybir.AxisListType.X,
                            op=mybir.AluOpType.add)
                        nc.scalar.activation(den[:], den[:],
                                             mybir.ActivationFunctionType.Copy,
                                             bias=1e-16)
                        rden = eh.tile([P, H], f32, tag="rden")
                        nc.vector.reciprocal(rden[:], den[:])
                        # message accumulation tail: contiguous 2x fold chain
                        # over the remaining SCHUNK slots (measured 1.69ns/el
                        # for a strided reduce vs 0.55ns/el for folds)
                        ns = SCHUNK
                        with nc.allow_low_precision(reason="bf16 edge math"):
                            while ns > 1:
                                k = ns // 2
                                nc.vector.tensor_tensor(
                                    out=msgb[:, :k * HC],
                                    in0=msgb[:, :k * HC],
                                    in1=msgb[:, (ns - k) * HC:ns * HC],
                                    op=mybir.AluOpType.add)
                                ns -= k
                        h_t = eh.tile([P, HC], f32, tag="h")
                        nc.vector.tensor_tensor(
                            out=h_t[:].rearrange("p (h c) -> p h c", c=C),
                            in0=msgb[:, 0:HC].rearrange("p (h c) -> p h c", c=C),
                            in1=rden[:, :, None].to_broadcast([P, H, C]),
                            op=mybir.AluOpType.mult)
                        nc.vector.tensor_add(h_t[:], h_t[:], s_t[:])
                        hb = eh.tile([P, HC], bf16, tag="hb")
                        nc.scalar.activation(hb[:], h_t[:],
                                             mybir.ActivationFunctionType.Relu)
                        tp = et.tile([P, HC], bf16, tag="tp")
                        for kb in range(KB1):
                            nc.tensor.transpose(
                                tp[:, kb * P:(kb + 1) * P],
                                hb[:, kb * P:(kb + 1) * P], ident[:])
                        hT_t = eT.tile([P, HC], bf16, tag="hT")
                        nc.scalar.copy(hT_t[:], tp[:])
                        hook(t, hT_t)

            # ---------------- phase sequencing -----------------------------
            # table first: edge-phase gathers depend on it, while the q,s
            # projections only gate each tile's DVE math
            l1_table()
            l1_qs()

            # ---------------- layer-2 projections + chunked AllGather ------
            with (
                tc.tile_pool(name="p2o", bufs=2) as p2o,
                tc.tile_pool(name="p2p", bufs=1, space="PSUM") as p2p,
            ):
                def l2_hook(t, hT_t):
                    ps = p2p.tile([P, 4 * HC], f32, tag="ps")
                    for kb in range(KB1):
                        for j0 in range(0, 4 * HC, HC):
                            nc.tensor.matmul(
                                ps[:, j0:j0 + HC],
                                lhsT=hT_t[:, kb * P:(kb + 1) * P],
                                rhs=w_all1[:].rearrange(
                                    "p (kb n) -> p kb n",
                                    n=4 * HC)[:, kb, j0:j0 + HC],
                                start=(kb == 0), stop=(kb == KB1 - 1))
                    o = p2o.tile([P, 4 * HC], bf16, tag="o")
                    evac(t, o[:], ps[:], "ball1", scalar_only=True)
                    rows = slice(t * P, (t + 1) * P)
                    nc.sync.dma_start(kvin2[rows, :], o[:, 0:2 * HC])
                    nc.scalar.dma_start(qsd[1][rows, :], o[:, 2 * HC:4 * HC])
                    if (t + 1) % TPG == 0:
                        g = t // TPG
                        cc = nc.gpsimd.collective_compute(
                            "AllGather", mybir.AluOpType.bypass,
                            replica_groups=rg,
                            ins=[kvin2[g * RPG:(g + 1) * RPG, :].opt()],
                            outs=[kvt1[g * NCORES * RPG:
                                       (g + 1) * NCORES * RPG, :].opt()])
                        # stripped again in _fix_ag_wait (walrus allows only
                        # one update on the collective); keeps the Tile
                        # scheduling sim's wait satisfiable.
                        cc.then_inc(agsem, 1)

                def fin_hook(t, hT_t):
                    ps = p2p.tile([P, D], f32, tag="psf")
                    for kb in range(KB1):
                        nc.tensor.matmul(
                            ps[:], lhsT=hT_t[:, kb * P:(kb + 1) * P],
                            rhs=w_out[:].rearrange(
                                "p (kb n) -> p kb n", n=D)[:, kb, :],
                            start=(kb == 0), stop=(kb == KB1 - 1))
                    o = p2o.tile([P, D], f32, tag="of")
                    evac(t, o[:], ps[:], "bout", scalar_only=True)
                    nc.sync.dma_start(out_d[t * P:(t + 1) * P, :], o[:])

                edge_phase(0, l2_hook)
                nc.gpsimd.wait_ge(agsem, G)
                edge_phase(1, fin_hook)

    _fix_ag_wait(nc, agsem.num)
    _split_waits(nc)
    return nc


def _fix_ag_wait(nc, agsem_num):
    """The AllGather->L2-gather ordering: walrus rejects a second sem
    update on the collective, so instead of incrementing our own agsem we
    rewrite the placeholder wait (on agsem) to wait on the framework's
    Collectives_* semaphore, which each AllGather bumps by 1 at
    completion."""
    cc_sem = None
    n_cc = 0
    for fn in nc.m.functions:
        for bb in fn.blocks:
            for ins in bb.instructions:
                if type(ins).__name__ == "InstCollectiveCompute":
                    n_cc += 1
                    si = ins.sync_info
                    for u in list(si.on_update):
                        if u.id == agsem_num:
                            si.on_update.remove(u)
                        elif u.ant_name and u.ant_name.startswith("Collectives"):
                            assert cc_sem is None or cc_sem == u.id
                            cc_sem = u.id
    assert cc_sem is not None and n_cc == G, (cc_sem, n_cc)
    patched = 0
    for fn in nc.m.functions:
        for bb in fn.blocks:
            for ins in bb.instructions:
                si = ins.sync_info
                if not si or not si.on_wait:
                    continue
                for w in si.on_wait:
                    if w.id == agsem_num:
                        w.id = cc_sem
                        w.ant_name = "Collectives_agfix"
                        w.wait_value = n_cc
                        patched += 1
    assert patched == 1, patched


# ---------------------------------------------------------------- host glue
def _bf16():
    import concourse.mybir as mybir
    return mybir.dt.np(mybir.dt.bfloat16)


def _make_in_maps(inputs, srcidx1, srcidx2, maskbH, node_of_rank,
                  slot_of_node, subsets, SUBPAD):
    bf = _bf16()
    x = np.asarray(inputs["x"], np.float32)
    g = lambda n: np.asarray(inputs[n], np.float32)

    # node at each table slot (slots not assigned to a node stay 0 and are
    # never referenced by a real edge)
    node_of_slot = np.zeros(NPAD, np.int64)
    node_of_slot[slot_of_node] = np.arange(N)
    # own shard, local order: core c, local j -> node_of_rank[j*8 + c]
    r = np.arange(N)
    xsh = np.zeros((NCORES, SHARD, D), np.float32)
    xsh[r % NCORES, r // NCORES] = x[node_of_rank]

    common = {
        "wkv0": np.concatenate([g("k0_w").T, g("v0_w").T], 1).astype(bf),
        "wqs0": np.concatenate([g("q0_w").T, g("s0_w").T], 1).astype(bf),
        "wall1": np.concatenate(
            [g("k1_w").T, g("v1_w").T, g("q1_w").T, g("s1_w").T], 1).astype(bf),
        "wout": np.ascontiguousarray(g("out_w").T).astype(bf),
        "bkv0": np.concatenate([g("k0_b"), g("v0_b")]).reshape(1, -1).astype(bf),
        "bqs0": np.concatenate([g("q0_b"), g("s0_b")]).reshape(1, -1).astype(bf),
        "ball1": np.concatenate(
            [g("k1_b"), g("v1_b"), g("q1_b"), g("s1_b")]).reshape(1, -1).astype(bf),
        "bout": g("out_b").reshape(1, -1).astype(bf),
    }
    in_maps = []
    for c in range(NCORES):
        m = dict(common)
        xs = np.zeros((SUBPAD, D), np.float32)
        xs[:len(subsets[c])] = x[node_of_slot[subsets[c]]]
        m["xTs"] = np.ascontiguousarray(xs.T).astype(bf)
        m["xTo"] = np.ascontiguousarray(xsh[c].T).astype(bf)
        m["srcidx1"] = np.ascontiguousarray(srcidx1[c])
        m["srcidx2"] = np.ascontiguousarray(srcidx2[c])
        m["maskb"] = np.ascontiguousarray(maskbH[c]).astype(bf)
        in_maps.append(m)
    return in_maps


def _biases_zero(inputs):
    return all(
        not np.any(np.asarray(inputs[nm]))
        for nm in [f"{p}{l}_b" for l in range(2) for p in "qkvs"] + ["out_b"])


def kernel(**inputs):
    from concourse.bass_utils import run_bass_kernel_spmd
    (srcidx1, srcidx2, maskbH, Dts, SUMD, coloff, node_of_rank,
     slot_of_node, subsets, SUBPAD) = _prep(np.asarray(inputs["edge_index"]))
    nc = _build_nc(Dts, SUMD, coloff, SUBPAD, bias_zero=_biases_zero(inputs))
    in_maps = _make_in_maps(inputs, srcidx1, srcidx2, maskbH, node_of_rank,
                            slot_of_node, subsets, SUBPAD)
    res = run_bass_kernel_spmd(nc, in_maps, core_ids=list(range(NCORES)))
    shards = np.stack([res.results[c]["out"] for c in range(NCORES)])
    full = np.empty((N, D), np.float32)
    r = np.arange(N)
    full[node_of_rank] = shards[r % NCORES, r // NCORES]
    return full


# revision 34
# speedup vs baseline: 1.2909x; 1.2909x over previous
"""GraphTransformer (2x PyG TransformerConv + out proj) on 8 trn2 NeuronCores.

v3 strategy (edge-parallel dst-ownership, DVE-lean edge math):
- Host: sort nodes by (degree, id); rank r -> core r%8, local r//8. Edges
  grouped by dst; per-tile max degree D_t is SPMD-uniform. Slot layout
  (group, core, local) makes the L2 AllGather chunkable into G groups.
- Layer 1 k|v table: each core builds kv only for the ~31.7k nodes its
  edges actually reference (host-compacted xTs input) -> smaller PE work
  and HBM traffic vs a full replicated table. Gather indices are
  subset-local.
- Layer 2: per-tile q,k,v,s projections inside the L1 edge loop; per-group
  bf16 AllGathers of the full k|v table overlap remaining L1 edge work.
- Edge phase per 128-dst tile: per-slot indirect gathers (bf16 kv row,
  2KB) round-robined over 2 SWDGE queues; logits via contiguous 2x-mode
  DVE ops (q broadcast in1, no materialized copy); additive -30000 mask
  bias pre-exp (robust to garbage in padded slots); exp weights expanded
  across C on the Act engine so the weighted-v product stays in DVE 2x
  mode with natural-layout writes; message accumulated per tile with one
  strided reduce (no transposed-write products, which measured 13-19us).
"""
import numpy as np

N, E, D, H, C, HC = 50000, 400000, 384, 4, 128, 512
NCORES, P = 8, 128
NLOC = N // NCORES
G = 7                     # collective groups (NTILES % G == 0)
NTILES = -(-((NLOC + P - 1) // P) // G) * G   # ceil to multiple of G
SHARD = NTILES * P
NPAD = SHARD * NCORES
SCHUNK = 8
NQ = 4                    # SWDGE queues for indirect gathers
TPG = NTILES // G         # tiles per group
RPG = TPG * P             # rows per group per core
INV_SQRT_C = 1.0 / np.sqrt(np.float32(C))
MASKNEG = -30000.0


# ---------------------------------------------------------------- host prep
def _prep(edge_index):
    src = np.asarray(edge_index[0], dtype=np.int64)
    dst = np.asarray(edge_index[1], dtype=np.int64)
    deg = np.bincount(dst, minlength=N)
    node_of_rank = np.lexsort((np.arange(N), deg))
    rank_of_node = np.empty(N, np.int64)
    rank_of_node[node_of_rank] = np.arange(N)
    # rank -> (core, local); local -> (group, j); table slot layout
    # s = (g*NCORES + c)*RPG + j  with g = local//RPG, j = local%RPG
    r = np.arange(N)
    core_r = r % NCORES
    local_r = r // NCORES
    slot_r = (local_r // RPG * NCORES + core_r) * RPG + local_r % RPG
    slot_of_node = np.empty(N, np.int64)
    slot_of_node[node_of_rank] = slot_r

    deg_sorted = deg[node_of_rank]
    Dts = []
    for t in range(NTILES):
        blk = deg_sorted[t * P * NCORES:(t + 1) * P * NCORES]
        Dts.append(max(int(blk.max()) if len(blk) else 0, 1))
    SUMD = sum(Dts)
    coloff = np.cumsum([0] + Dts)[:-1]

    er = rank_of_node[dst]
    order = np.argsort(er, kind="stable")
    er_s = er[order]
    gslot_s = slot_of_node[src[order]]
    starts = np.searchsorted(er_s, np.arange(N))
    slot = np.arange(E) - starts[er_s]

    core_e = er_s % NCORES
    local_e = er_s // NCORES
    col_e = coloff[local_e // P] + slot
    p_e = local_e % P

    srcidx1 = np.zeros((NCORES, P, SUMD), np.int32)   # subset-local (L1)
    srcidx2 = np.zeros((NCORES, P, SUMD), np.int32)   # global slot (L2)
    maskb = np.full((NCORES, P, SUMD), MASKNEG, np.float32)
    subsets = []
    for c in range(NCORES):
        m = core_e == c
        u = np.unique(gslot_s[m])
        sub_of = np.zeros(NPAD, np.int32)
        sub_of[u] = np.arange(len(u), dtype=np.int32)
        srcidx1[c, p_e[m], col_e[m]] = sub_of[gslot_s[m]]
        srcidx2[c, p_e[m], col_e[m]] = gslot_s[m]
        maskb[c, p_e[m], col_e[m]] = 0.0
        subsets.append(u)
    SUBPAD = -(-max(len(u) for u in subsets) // 1024) * 1024
    maskbH = np.repeat(maskb, H, axis=2)              # (s h) layout
    return (srcidx1, srcidx2, maskbH, Dts, SUMD, coloff,
            node_of_rank, slot_of_node, subsets, SUBPAD)


# ---------------------------------------------------------------- wait fix
def _split_waits(nc):
    """walrus here rejects >1 sem-wait per instruction; split extras onto
    InstNoOp carriers inserted just before, same engine."""
    import concourse.mybir as mybir
    for fn in nc.m.functions:
        for bb in fn.blocks:
            out = []
            changed = False
            for ins in bb.instructions:
                si = ins.sync_info
                waits = list(si.on_wait) if si and si.on_wait else []
                if len(waits) > 1:
                    changed = True
                    for j, w in enumerate(waits[:-1]):
                        out.append(mybir.InstNoOp(
                            name=f"{ins.name}-wf{j}", opcode="NoOp",
                            engine=ins.engine,
                            sync_info=mybir.SyncInfo(on_wait=[w], on_update=[]),
                            text_hint="waitfix"))
                    si.on_wait = waits[-1:]
                out.append(ins)
            if changed:
                bb.instructions = out


# ---------------------------------------------------------------- bass build
def _build_nc(Dts, SUMD, coloff, SUBPAD, bias_zero=False):
    import concourse.bass as bass
    import concourse.mybir as mybir
    import concourse.tile as tile
    from concourse.masks import make_identity
    f32 = mybir.dt.float32
    bf16 = mybir.dt.bfloat16

    nc = bass.Bass(num_devices=NCORES, num_swdge_queues=NQ)
    # L2 k|v table: raw Shared tensor so G chunked AllGathers can write
    # disjoint row ranges (pool tiles enforce a single writer). Ordering vs
    # the L2 gathers is by a manual semaphore incremented per AG.
    kvt1 = nc.dram_tensor("kvt1buf", [NPAD, 2 * HC], bf16,
                          kind="Internal", addr_space="Shared")
    agsem = nc.alloc_semaphore("agsem")
    xTs = nc.dram_tensor("xTs", [D, SUBPAD], bf16, kind="ExternalInput")
    xTo = nc.dram_tensor("xTo", [D, SHARD], bf16, kind="ExternalInput")
    srcidx1_d = nc.dram_tensor("srcidx1", [P, SUMD], mybir.dt.int32, kind="ExternalInput")
    srcidx2_d = nc.dram_tensor("srcidx2", [P, SUMD], mybir.dt.int32, kind="ExternalInput")
    maskb_d = nc.dram_tensor("maskb", [P, SUMD * H], bf16, kind="ExternalInput")
    wkv0 = nc.dram_tensor("wkv0", [D, 2 * HC], bf16, kind="ExternalInput")
    wqs0 = nc.dram_tensor("wqs0", [D, 2 * HC], bf16, kind="ExternalInput")
    wall1 = nc.dram_tensor("wall1", [HC, 4 * HC], bf16, kind="ExternalInput")
    wout = nc.dram_tensor("wout", [HC, D], bf16, kind="ExternalInput")
    bkv0 = nc.dram_tensor("bkv0", [1, 2 * HC], bf16, kind="ExternalInput")
    bqs0 = nc.dram_tensor("bqs0", [1, 2 * HC], bf16, kind="ExternalInput")
    ball1 = nc.dram_tensor("ball1", [1, 4 * HC], bf16, kind="ExternalInput")
    bout = nc.dram_tensor("bout", [1, D], bf16, kind="ExternalInput")
    out_d = nc.dram_tensor("out", [SHARD, D], f32, kind="ExternalOutput")

    chunks = []  # per tile: list of (coloff, S)
    for t in range(NTILES):
        cs, off = [], 0
        while off < Dts[t]:
            cs.append((int(coloff[t]) + off, min(SCHUNK, Dts[t] - off)))
            off += SCHUNK
        chunks.append(cs)
    ncmax = max(len(c) for c in chunks)

    KB0 = D // P   # 3
    KB1 = HC // P  # 4
    rg = [list(range(NCORES))]
    qrot = [0]

    def igather(out_ap, tab, off_ap):
        bi = nc.gpsimd.indirect_dma_start(
            out=out_ap, out_offset=None, in_=tab,
            in_offset=bass.IndirectOffsetOnAxis(ap=off_ap, axis=0))
        q = qrot[0] % NQ
        qrot[0] += 1
        if q:
            bi.ins.queue = f"qPoolDynamic{q}"
        return bi

    with tile.TileContext(nc) as tc:
        with (
            tc.tile_pool(name="dram", bufs=1, space="DRAM") as dram,
            tc.tile_pool(name="const", bufs=1) as const,
        ):
            kvt0 = dram.tile([SUBPAD, 2 * HC], bf16, name="kvt0")
            qsd = [dram.tile([SHARD, 2 * HC], bf16, name=f"qs{l}d")
                   for l in range(2)]
            kvin2 = dram.tile([SHARD, 2 * HC], bf16, name="kvin2")

            nc.gpsimd.sem_clear(range(agsem.num, agsem.num + 1))
            ident = const.tile([P, P], bf16)
            make_identity(nc, ident[:])
            ones = const.tile([1, P], bf16)
            nc.vector.memset(ones[:], 1.0)
            srcidx1_s = const.tile([P, SUMD], mybir.dt.int32)
            nc.sync.dma_start(srcidx1_s[:], srcidx1_d[:])
            srcidx2_s = const.tile([P, SUMD], mybir.dt.int32)
            nc.sync.dma_start(srcidx2_s[:], srcidx2_d[:])
            maskb_s = const.tile([P, SUMD * H], bf16)
            nc.sync.dma_start(maskb_s[:], maskb_d[:])

            w_kv0 = const.tile([P, KB0 * 2 * HC], bf16)
            nc.sync.dma_start(
                w_kv0[:].rearrange("p (kb n) -> p kb n", n=2 * HC),
                wkv0[:].rearrange("(kb p) n -> p kb n", p=P))
            w_qs0 = const.tile([P, KB0 * 2 * HC], bf16)
            nc.sync.dma_start(
                w_qs0[:].rearrange("p (kb n) -> p kb n", n=2 * HC),
                wqs0[:].rearrange("(kb p) n -> p kb n", p=P))
            w_all1 = const.tile([P, KB1 * 4 * HC], bf16)
            nc.sync.dma_start(
                w_all1[:].rearrange("p (kb n) -> p kb n", n=4 * HC),
                wall1[:].rearrange("(kb p) n -> p kb n", p=P))
            w_out = const.tile([P, KB1 * D], bf16)
            nc.sync.dma_start(
                w_out[:].rearrange("p (kb n) -> p kb n", n=D),
                wout[:].rearrange("(kb p) n -> p kb n", p=P))

            # biases, broadcast across partitions via ones-matmul (PE)
            bsrc = {}
            for nm, dt_, wid in (("bkv0", bkv0, 2 * HC), ("bqs0", bqs0, 2 * HC),
                                 ("ball1", ball1, 4 * HC), ("bout", bout, D)):
                t_ = const.tile([1, wid], bf16, name=f"{nm}row")
                nc.sync.dma_start(t_[:], dt_[:])
                bsrc[nm] = t_
            bb = {}
            with tc.tile_pool(name="bbp", bufs=1, space="PSUM") as bbp:
                psb = bbp.tile([P, 4 * HC], f32, tag="psb")
                for nm, wid in (("bkv0", 2 * HC), ("bqs0", 2 * HC),
                                ("ball1", 4 * HC), ("bout", D)):
                    for j0 in range(0, wid, HC):
                        j1 = min(j0 + HC, wid)
                        nc.tensor.matmul(psb[:, j0:j1], lhsT=ones[:1, :],
                                         rhs=bsrc[nm][:1, j0:j1],
                                         start=True, stop=True)
                    bb[nm] = const.tile([P, wid], bf16, name=f"bb{nm}")
                    nc.scalar.copy(bb[nm][:], psb[:, :wid])

            def evac(idx, o, ps, bname, scalar_only=False):
                """PSUM -> SBUF with bias add. gpsimd can't read PSUM on
                hw; alternate Act/DVE in PE-bound phases, Act-only during
                the edge phases (DVE is the bottleneck there)."""
                if bias_zero:
                    if scalar_only or idx % 2 == 0:
                        nc.scalar.copy(o, ps)
                    else:
                        nc.vector.tensor_copy(o, ps)
                else:
                    nc.vector.tensor_tensor(out=o, in0=ps, in1=bb[bname][:],
                                            op=mybir.AluOpType.add)

            # ---------------- layer-1 q,s projections (own shard) ----------
            def l1_qs():
                with (
                    tc.tile_pool(name="qsl", bufs=6) as lp,
                    tc.tile_pool(name="qso", bufs=2) as op,
                    tc.tile_pool(name="qsp", bufs=2, space="PSUM") as pp,
                ):
                    for g8 in range((NTILES + 7) // 8):
                        lhs = []
                        t0 = g8 * 8
                        nt = min(8, NTILES - t0)
                        for kb in range(KB0):
                            lt = lp.tile([P, 8 * P], bf16, tag="lhs")
                            nc.sync.dma_start(
                                lt[:, :nt * P],
                                xTo[kb * P:(kb + 1) * P, t0 * P:(t0 + nt) * P])
                            lhs.append(lt)
                        for ti in range(nt):
                            t = t0 + ti
                            ps = pp.tile([P, 2 * HC], f32, tag="ps")
                            for kb in range(KB0):
                                for j0 in range(0, 2 * HC, HC):
                                    nc.tensor.matmul(
                                        ps[:, j0:j0 + HC],
                                        lhsT=lhs[kb][:, ti * P:(ti + 1) * P],
                                        rhs=w_qs0[:].rearrange(
                                            "p (kb n) -> p kb n",
                                            n=2 * HC)[:, kb, j0:j0 + HC],
                                        start=(kb == 0), stop=(kb == KB0 - 1))
                            o = op.tile([P, 2 * HC], bf16, tag="o")
                            evac(t, o[:], ps[:], "bqs0")
                            rows = slice(t * P, (t + 1) * P)
                            nc.scalar.dma_start(qsd[0][rows, :], o[:])

            # ---------------- layer-1 subset k|v table ---------------------
            def l1_table():
                NT_F = SUBPAD // P
                with (
                    tc.tile_pool(name="tbl", bufs=6) as lp,
                    tc.tile_pool(name="tbo", bufs=3) as op,
                    tc.tile_pool(name="tbp", bufs=3, space="PSUM") as pp,
                ):
                    for g8 in range(NT_F // 8):
                        lhs = []
                        t0 = g8 * 8
                        for kb in range(KB0):
                            lt = lp.tile([P, 8 * P], bf16, tag="lhs")
                            nc.sync.dma_start(
                                lt[:], xTs[kb * P:(kb + 1) * P, t0 * P:(t0 + 8) * P])
                            lhs.append(lt)
                        for ti in range(8):
                            u = t0 + ti
                            ps = pp.tile([P, 2 * HC], f32, tag="ps")
                            for kb in range(KB0):
                                for j0 in range(0, 2 * HC, HC):
                                    nc.tensor.matmul(
                                        ps[:, j0:j0 + HC],
                                        lhsT=lhs[kb][:, ti * P:(ti + 1) * P],
                                        rhs=w_kv0[:].rearrange(
                                            "p (kb n) -> p kb n",
                                            n=2 * HC)[:, kb, j0:j0 + HC],
                                        start=(kb == 0), stop=(kb == KB0 - 1))
                            o = op.tile([P, 2 * HC], bf16, tag="o")
                            evac(u, o[:], ps[:], "bkv0")
                            qdma = nc.sync if u % 2 == 0 else nc.scalar
                            qdma.dma_start(kvt0[u * P:(u + 1) * P, :], o[:])

            # ---------------- edge phase (layer l), single pass ------------
            def edge_phase(l, hook):
                tab = kvt0 if l == 0 else kvt1
                sidx = srcidx1_s if l == 0 else srcidx2_s
                with (
                    tc.tile_pool(name=f"eg{l}", bufs=3) as eg,
                    tc.tile_pool(name=f"ew{l}", bufs=2) as ew,
                    tc.tile_pool(name=f"eh{l}", bufs=2) as eh,
                    tc.tile_pool(name=f"eT{l}", bufs=2) as eT,
                    tc.tile_pool(name=f"et{l}", bufs=2, space="PSUM") as et,
                ):
                    for t in range(NTILES):
                        rows = slice(t * P, (t + 1) * P)
                        Dt = Dts[t]
                        nch = len(chunks[t])
                        q_t = ew.tile([P, HC], bf16, tag="q")
                        nc.sync.dma_start(q_t[:], qsd[l][rows, 0:HC])
                        s_t = ew.tile([P, HC], bf16, tag="s")
                        nc.scalar.dma_start(s_t[:], qsd[l][rows, HC:2 * HC])
                        msgb = eh.tile([P, ncmax * SCHUNK * HC], bf16, tag="msgb")
                        e_all = eh.tile([P, ncmax * SCHUNK * H], bf16, tag="eall")
                        nc.vector.memset(e_all[:], 0.0)
                        for ci, (co, S) in enumerate(chunks[t]):
                            cb = ci * SCHUNK
                            kvg = eg.tile([P, SCHUNK * 2 * HC], bf16, tag="kvg")
                            if t < 3:
                                # first use of each ring buffer: stale SBUF
                                # bits could be NaN; NaN*0 != 0 in the padded
                                # ev slots below
                                nc.vector.memset(kvg[:], 0.0)
                            kv3 = kvg[:].rearrange("p (s kv) -> p s kv", kv=2 * HC)
                            for s in range(S):
                                igather(kv3[:, s, :], tab[:],
                                        sidx[:, co + s:co + s + 1])
                            # q*k product lands in msgb's chunk columns (later
                            # overwritten by the weighted-v product)
                            prod = msgb[:, cb * HC:(cb + SCHUNK) * HC]
                            nc.vector.tensor_tensor(
                                out=prod.rearrange(
                                    "p (s n) -> p s n", n=HC)[:, :S],
                                in0=kv3[:, :S, 0:HC],
                                in1=q_t[:, None, :].to_broadcast([P, S, HC]),
                                op=mybir.AluOpType.mult)
                            # logit reduction over C: two 2x contiguous folds
                            # (128->64->32) + one 1x reduce of 32. The fold
                            # scratch aliases the eexp tile (dead until the
                            # broadcast-copy below, which runs after the TR).
                            eexp = eT.tile([P, SCHUNK * HC], bf16, tag="eexp")
                            p3 = prod.rearrange("p (sh c) -> p sh c", c=C)
                            f3 = eexp[:, :SCHUNK * H * 64].rearrange(
                                "p (sh c) -> p sh c", c=64)
                            with nc.allow_low_precision(reason="bf16 edge math"):
                                nc.vector.tensor_tensor(
                                    out=f3[:, :S * H], in0=p3[:, :S * H, 0:64],
                                    in1=p3[:, :S * H, 64:128],
                                    op=mybir.AluOpType.add)
                                nc.vector.tensor_tensor(
                                    out=f3[:, :S * H, 0:32],
                                    in0=f3[:, :S * H, 0:32],
                                    in1=f3[:, :S * H, 32:64],
                                    op=mybir.AluOpType.add)
                                am = eT.tile([P, SCHUNK * H], bf16, tag="am")
                                nc.vector.tensor_reduce(
                                    am[:, :S * H],
                                    f3[:, :S * H, 0:32],
                                    axis=mybir.AxisListType.X,
                                    op=mybir.AluOpType.add)
                            nc.vector.tensor_tensor(
                                out=am[:, :S * H], in0=am[:, :S * H],
                                in1=maskb_s[:, (co) * H:(co + S) * H],
                                op=mybir.AluOpType.add)
                            nc.scalar.activation(
                                e_all[:, cb * H:cb * H + S * H],
                                am[:, :S * H],
                                mybir.ActivationFunctionType.Exp,
                                scale=float(INV_SQRT_C))
                            # e expanded across C on Act (has slack) so the
                            # weighted-v product stays a packed 2x DVE op
                            nc.scalar.copy(
                                eexp[:].rearrange(
                                    "p (s h c) -> p s h c", h=H, c=C),
                                e_all[:, cb * H:(cb + SCHUNK) * H]
                                .rearrange("p (s h) -> p s h", h=H)
                                [:, :, :, None].to_broadcast(
                                    [P, SCHUNK, H, C]))
                            # full SCHUNK width: padded slots have e==0 so the
                            # stale k|v garbage is zeroed, keeping the fold
                            # chain below valid
                            nc.vector.tensor_tensor(
                                out=msgb[:, cb * HC:(cb + SCHUNK) * HC],
                                in0=kv3[:, :, HC:2 * HC],
                                in1=eexp[:],
                                op=mybir.AluOpType.mult)
                            if ci > 0:
                                # fold this chunk's slots into block 0 now so
                                # the tile tail only folds one chunk's worth
                                with nc.allow_low_precision(
                                        reason="bf16 edge math"):
                                    nc.vector.tensor_tensor(
                                        out=msgb[:, :SCHUNK * HC],
                                        in0=msgb[:, :SCHUNK * HC],
                                        in1=msgb[:, cb * HC:
                                                 (cb + SCHUNK) * HC],
                                        op=mybir.AluOpType.add)
                        den = eh.tile([P, H], f32, tag="den")
                        nc.vector.tensor_reduce(
                            den[:],
                            e_all[:, :Dt * H].rearrange(
                                "p (s h) -> p h s", h=H),
                            axis=mybir.AxisListType.X,
                            op=mybir.AluOpType.add)
                        nc.scalar.activation(den[:], den[:],
                                             mybir.ActivationFunctionType.Copy,
                                             bias=1e-16)
                        rden = eh.tile([P, H], f32, tag="rden")
                        nc.vector.reciprocal(rden[:], den[:])
                        # message accumulation tail: contiguous 2x fold chain
                        # over the remaining SCHUNK slots (measured 1.69ns/el
                        # for a strided reduce vs 0.55ns/el for folds)
                        ns = SCHUNK
                        with nc.allow_low_precision(reason="bf16 edge math"):
                            while ns > 1:
                                k = ns // 2
                                nc.vector.tensor_tensor(
                                    out=msgb[:, :k * HC],
                                    in0=msgb[:, :k * HC],
                                    in1=msgb[:, (ns - k) * HC:ns * HC],
                                    op=mybir.AluOpType.add)
                                ns -= k
                        h_t = eh.tile([P, HC], f32, tag="h")
                        nc.vector.tensor_tensor(
                            out=h_t[:].rearrange("p (h c) -> p h c", c=C),
                            in0=msgb[:, 0:HC].rearrange("p (h c) -> p h c", c=C),
                            in1=rden[:, :, None].to_broadcast([P, H, C]),
                            op=mybir.AluOpType.mult)
                        nc.vector.tensor_add(h_t[:], h_t[:], s_t[:])
                        hb = eh.tile([P, HC], bf16, tag="hb")
                        nc.scalar.activation(hb[:], h_t[:],
                                             mybir.ActivationFunctionType.Relu)
                        tp = et.tile([P, HC], bf16, tag="tp")
                        for kb in range(KB1):
                            nc.tensor.transpose(
                                tp[:, kb * P:(kb + 1) * P],
                                hb[:, kb * P:(kb + 1) * P], ident[:])
                        hT_t = eT.tile([P, HC], bf16, tag="hT")
                        nc.scalar.copy(hT_t[:], tp[:])
                        hook(t, hT_t)

            # ---------------- phase sequencing -----------------------------
            # table first: edge-phase gathers depend on it, while the q,s
            # projections only gate each tile's DVE math
            l1_table()
            l1_qs()

            # ---------------- layer-2 projections + chunked AllGather ------
            with (
                tc.tile_pool(name="p2o", bufs=2) as p2o,
                tc.tile_pool(name="p2p", bufs=1, space="PSUM") as p2p,
            ):
                def l2_hook(t, hT_t):
                    ps = p2p.tile([P, 4 * HC], f32, tag="ps")
                    for kb in range(KB1):
                        for j0 in range(0, 4 * HC, HC):
                            nc.tensor.matmul(
                                ps[:, j0:j0 + HC],
                                lhsT=hT_t[:, kb * P:(kb + 1) * P],
                                rhs=w_all1[:].rearrange(
                                    "p (kb n) -> p kb n",
                                    n=4 * HC)[:, kb, j0:j0 + HC],
                                start=(kb == 0), stop=(kb == KB1 - 1))
                    o = p2o.tile([P, 4 * HC], bf16, tag="o")
                    evac(t, o[:], ps[:], "ball1", scalar_only=True)
                    rows = slice(t * P, (t + 1) * P)
                    nc.sync.dma_start(kvin2[rows, :], o[:, 0:2 * HC])
                    nc.scalar.dma_start(qsd[1][rows, :], o[:, 2 * HC:4 * HC])
                    if (t + 1) % TPG == 0:
                        g = t // TPG
                        cc = nc.gpsimd.collective_compute(
                            "AllGather", mybir.AluOpType.bypass,
                            replica_groups=rg,
                            ins=[kvin2[g * RPG:(g + 1) * RPG, :].opt()],
                            outs=[kvt1[g * NCORES * RPG:
                                       (g + 1) * NCORES * RPG, :].opt()])
                        # stripped again in _fix_ag_wait (walrus allows only
                        # one update on the collective); keeps the Tile
                        # scheduling sim's wait satisfiable.
                        cc.then_inc(agsem, 1)

                def fin_hook(t, hT_t):
                    ps = p2p.tile([P, D], f32, tag="psf")
                    for kb in range(KB1):
                        nc.tensor.matmul(
                            ps[:], lhsT=hT_t[:, kb * P:(kb + 1) * P],
                            rhs=w_out[:].rearrange(
                                "p (kb n) -> p kb n", n=D)[:, kb, :],
                            start=(kb == 0), stop=(kb == KB1 - 1))
                    o = p2o.tile([P, D], f32, tag="of")
                    evac(t, o[:], ps[:], "bout", scalar_only=True)
                    nc.sync.dma_start(out_d[t * P:(t + 1) * P, :], o[:])

                edge_phase(0, l2_hook)
                nc.gpsimd.wait_ge(agsem, G)
                edge_phase(1, fin_hook)

    _fix_ag_wait(nc, agsem.num)
    _split_waits(nc)
    return nc


def _fix_ag_wait(nc, agsem_num):
    """The AllGather->L2-gather ordering: walrus rejects a second sem
    update on the collective, so instead of incrementing our own agsem we
    rewrite the placeholder wait (on agsem) to wait on the framework's
    Collectives_* semaphore, which each AllGather bumps by 1 at
    completion."""
    cc_sem = None
    n_cc = 0
    for fn in nc.m.functions:
        for bb in fn.blocks:
            for ins in bb.instructions:
                if type(ins).__name__ == "InstCollectiveCompute":
                    n_cc += 1
                    si = ins.sync_info
                    for u in list(si.on_update):
                        if u.id == agsem_num:
                            si.on_update.remove(u)
                        elif u.ant_name and u.ant_name.startswith("Collectives"):
                            assert cc_sem is None or cc_sem == u.id
                            cc_sem = u.id
    assert cc_sem is not None and n_cc == G, (cc_sem, n_cc)
    patched = 0
    for fn in nc.m.functions:
        for bb in fn.blocks:
            for ins in bb.instructions:
                si = ins.sync_info
                if not si or not si.on_wait:
                    continue
                for w in si.on_wait:
                    if w.id == agsem_num:
                        w.id = cc_sem
                        w.ant_name = "Collectives_agfix"
                        w.wait_value = n_cc
                        patched += 1
    assert patched == 1, patched


# ---------------------------------------------------------------- host glue
def _bf16():
    import concourse.mybir as mybir
    return mybir.dt.np(mybir.dt.bfloat16)


def _make_in_maps(inputs, srcidx1, srcidx2, maskbH, node_of_rank,
                  slot_of_node, subsets, SUBPAD):
    bf = _bf16()
    x = np.asarray(inputs["x"], np.float32)
    g = lambda n: np.asarray(inputs[n], np.float32)

    # node at each table slot (slots not assigned to a node stay 0 and are
    # never referenced by a real edge)
    node_of_slot = np.zeros(NPAD, np.int64)
    node_of_slot[slot_of_node] = np.arange(N)
    # own shard, local order: core c, local j -> node_of_rank[j*8 + c]
    r = np.arange(N)
    xsh = np.zeros((NCORES, SHARD, D), np.float32)
    xsh[r % NCORES, r // NCORES] = x[node_of_rank]

    common = {
        "wkv0": np.concatenate([g("k0_w").T, g("v0_w").T], 1).astype(bf),
        "wqs0": np.concatenate([g("q0_w").T, g("s0_w").T], 1).astype(bf),
        "wall1": np.concatenate(
            [g("k1_w").T, g("v1_w").T, g("q1_w").T, g("s1_w").T], 1).astype(bf),
        "wout": np.ascontiguousarray(g("out_w").T).astype(bf),
        "bkv0": np.concatenate([g("k0_b"), g("v0_b")]).reshape(1, -1).astype(bf),
        "bqs0": np.concatenate([g("q0_b"), g("s0_b")]).reshape(1, -1).astype(bf),
        "ball1": np.concatenate(
            [g("k1_b"), g("v1_b"), g("q1_b"), g("s1_b")]).reshape(1, -1).astype(bf),
        "bout": g("out_b").reshape(1, -1).astype(bf),
    }
    in_maps = []
    for c in range(NCORES):
        m = dict(common)
        xs = np.zeros((SUBPAD, D), np.float32)
        xs[:len(subsets[c])] = x[node_of_slot[subsets[c]]]
        m["xTs"] = np.ascontiguousarray(xs.T).astype(bf)
        m["xTo"] = np.ascontiguousarray(xsh[c].T).astype(bf)
        m["srcidx1"] = np.ascontiguousarray(srcidx1[c])
        m["srcidx2"] = np.ascontiguousarray(srcidx2[c])
        m["maskb"] = np.ascontiguousarray(maskbH[c]).astype(bf)
        in_maps.append(m)
    return in_maps


def _biases_zero(inputs):
    return all(
        not np.any(np.asarray(inputs[nm]))
        for nm in [f"{p}{l}_b" for l in range(2) for p in "qkvs"] + ["out_b"])


def kernel(**inputs):
    from concourse.bass_utils import run_bass_kernel_spmd
    (srcidx1, srcidx2, maskbH, Dts, SUMD, coloff, node_of_rank,
     slot_of_node, subsets, SUBPAD) = _prep(np.asarray(inputs["edge_index"]))
    nc = _build_nc(Dts, SUMD, coloff, SUBPAD, bias_zero=_biases_zero(inputs))
    in_maps = _make_in_maps(inputs, srcidx1, srcidx2, maskbH, node_of_rank,
                            slot_of_node, subsets, SUBPAD)
    res = run_bass_kernel_spmd(nc, in_maps, core_ids=list(range(NCORES)))
    shards = np.stack([res.results[c]["out"] for c in range(NCORES)])
    full = np.empty((N, D), np.float32)
    r = np.arange(N)
    full[node_of_rank] = shards[r % NCORES, r // NCORES]
    return full
